# revision 1
# baseline (speedup 1.0000x reference)
"""Trainium2 Bass kernel for nn_BaseLayerGate (MoE balanced routing).

8 NeuronCores, data-parallel over tokens:
  - Each core owns a 2048-token shard. Affinity matmul aff^T = centT.T @ featsT
    on the tensor engine (fp32): col-major aff^T [128 (2 slots x 64 experts), 2048].
  - Sinkhorn (10 iters) in reciprocal-potential form:
      R_sum[n]  = sum_se E0[n,se] * V[se]      (PE matvec, V = slot-masked 1/C_sum)
      C_sum[se] = sum_n  E0[n,se] * W[n]       (PE matvec accum, W = 1/R_sum)
    The token-direction sum is global: per-expert partials are exchanged with an
    AllGather of a [1,128] row (PE-transposed so both exchange DMAs are
    contiguous), summed on-chip. 10 R-steps, 9 C-steps/exchanges (the 10th
    C-step is a uniform per-column shift and cannot change top-k ordering).
  - Z^T = aff^T - ln(R_sum) broadcast (ACT Ln + one Newton step for the LUT),
    per-column ordering of Z equals the reference's final ordering.
"""

import numpy as np

import concourse.bass as bass
from concourse import mybir
from concourse.bass_utils import run_bass_kernel_spmd

N_CORES = 8
N = 16384
D = 1024
KSLOT = 2
E = 64
SE = KSLOT * E
CAP = N // E
TOK = N // N_CORES
ITERS = 10

F32 = mybir.dt.float32

# ---- semaphore schedule ----------------------------------------------------
# in_sem: centT(8) + v0 + ident + ones = 11 transfers; fsem[k]: featsT chunk k
# out_sem: afft + zt outputs
# dma_sem (all x16): exchange cc_in(it) -> 2it+1, gath(it) -> 2it+2; rflat +2
D_EXCH_END = 2 * (ITERS - 1)                 # 18
D_RFLAT = D_EXCH_END + 2                     # 20
# pe_sem: 1 aff | 2 R(0) | 3..18 transposes | C(it)=19+2it, R(it>=1)=18+2it
def P_R(it):
    return 2 if it == 0 else 18 + 2 * it
def P_C(it):
    return 19 + 2 * it
P_LAST_R = P_R(ITERS - 1)                     # 36
P_ZB = P_LAST_R + 3                           # 39
# act_sem: 1 aff-copy | 2 exp | e0tm copy t -> t+3 (-> 18) | Ln | texp | rT x2
A_E0TM = 18
A_LN = A_E0TM + 1                             # 19
A_TEXP = A_LN + 1                             # 20
A_RT = A_TEXP + 2                             # 22
# dve_sem: 1 affT copy | per it<9: W=2+2it, VU=3+2it | u, rl2, sub
def V_W(it):
    return 4 * it + 1
def V_EX(it):
    return 4 * it + 2
def V_RD(it):
    return 4 * it + 3
def V_VU(it):
    return 4 * it + 4
V_U = 4 * (ITERS - 1) + 1                     # 37
V_RL2 = V_U + 1                               # 38
V_SUB = V_RL2 + 1                             # 39


def _build_nc():
    nc = bass.Bass()

    featsT_in = nc.declare_dram_parameter("featsT", [D, TOK], F32, isOutput=False)
    centT_in = nc.declare_dram_parameter("centT", [D, SE], F32, isOutput=False)
    v0_in = nc.declare_dram_parameter("v0", [SE, 2], F32, isOutput=False)
    ident_in = nc.declare_dram_parameter("ident", [128, 128], F32, isOutput=False)
    ones_in = nc.declare_dram_parameter("ones", [1, 64], F32, isOutput=False)
    onesc_in = nc.declare_dram_parameter("onesc", [128, 1], F32, isOutput=False)

    zt_out = nc.declare_dram_parameter("zt", [SE, TOK], F32, isOutput=True)
    aff_out = nc.declare_dram_parameter("afft", [SE, TOK], F32, isOutput=True)

    cc_in = nc.dram_tensor("cc_in", [SE, 1], F32)
    cc_out = nc.dram_tensor("cc_out", [N_CORES * SE, 1], F32, addr_space="Shared")

    core_ids = list(range(N_CORES))

    from contextlib import ExitStack
    es = ExitStack()
    featsT_sb = es.enter_context(nc.sbuf_tensor("featsT_sb", [128, 8, TOK], F32))
    centT_sb = es.enter_context(nc.sbuf_tensor("centT_sb", [128, 8, SE], F32))
    affT_sb = es.enter_context(nc.sbuf_tensor("affT_sb", [128, TOK], F32))
    e0t_sb = es.enter_context(nc.sbuf_tensor("e0t_sb", [128, TOK], F32))
    e0tm_sb = es.enter_context(nc.sbuf_tensor("e0tm_sb", [128, 16, 128], F32))
    ident_sb = es.enter_context(nc.sbuf_tensor("ident_sb", [128, 128], F32))
    v_sb = es.enter_context(nc.sbuf_tensor("v_sb", [128, 2], F32))
    w_sb = es.enter_context(nc.sbuf_tensor("w_sb", [128, 16, 2], F32))
    cpart_sb = es.enter_context(nc.sbuf_tensor("cpart_sb", [128, 1], F32))
    crow_sb = es.enter_context(nc.sbuf_tensor("crow_sb", [1, SE], F32))
    gath_sb = es.enter_context(nc.sbuf_tensor("gath_sb", [128, SE], F32))
    csum_sb = es.enter_context(nc.sbuf_tensor("csum_sb", [128, 1], F32))
    g8_sb = es.enter_context(nc.sbuf_tensor("g8_sb", [128, 8], F32))
    rlog_sb = es.enter_context(nc.sbuf_tensor("rlog_sb", [128, 16, 2], F32))
    texp_sb = es.enter_context(nc.sbuf_tensor("texp_sb", [128, 32], F32))
    u_sb = es.enter_context(nc.sbuf_tensor("u_sb", [128, 32], F32))
    rlog2_sb = es.enter_context(nc.sbuf_tensor("rlog2_sb", [128, 16, 2], F32))
    rt_sb = es.enter_context(nc.sbuf_tensor("rt_sb", [16, 2, 128], F32))
    rflat0_sb = es.enter_context(nc.sbuf_tensor("rflat0_sb", [1, TOK], F32))
    rflat1_sb = es.enter_context(nc.sbuf_tensor("rflat1_sb", [1, TOK], F32))
    ones_sb = es.enter_context(nc.sbuf_tensor("ones_sb", [1, 64], F32))
    onesc_sb = es.enter_context(nc.sbuf_tensor("onesc_sb", [128, 1], F32))
    zt_sb = es.enter_context(nc.sbuf_tensor("zt_sb", [128, TOK], F32))
    ps_aff = es.enter_context(nc.psum_tensor("ps_aff", [128, TOK], F32))
    ps_tp = es.enter_context(nc.psum_tensor("ps_tp", [128, 512], F32))
    ps_r = es.enter_context(nc.psum_tensor("ps_r", [128, 512], F32))
    ps_c = es.enter_context(nc.psum_tensor("ps_c", [128, 512], F32))
    ps_tp2 = es.enter_context(nc.psum_tensor("ps_tp2", [128, 512], F32))
    block = es.enter_context(nc.Block())
    dma_sem = es.enter_context(nc.semaphore("dma_sem"))
    in_sem = es.enter_context(nc.semaphore("in_sem"))
    out_sem = es.enter_context(nc.semaphore("out_sem"))
    fsems = [es.enter_context(nc.semaphore(f"fsem{k}")) for k in range(8)]
    pe_sem = es.enter_context(nc.semaphore("pe_sem"))
    act_sem = es.enter_context(nc.semaphore("act_sem"))
    dve_sem = es.enter_context(nc.semaphore("dve_sem"))
    cc_sem = es.enter_context(nc.semaphore("cc_sem"))
    with es:
        # ---------------- sync engine: all DMA ----------------
        @block.sync
        def _(eng):
            for k in range(8):
                eng.dma_start(
                    out=centT_sb[:, k, :], in_=centT_in[128 * k : 128 * (k + 1), :]
                ).then_inc(in_sem, 16)
            eng.dma_start(out=v_sb[:], in_=v0_in[:]).then_inc(in_sem, 16)
            eng.dma_start(out=ident_sb[:], in_=ident_in[:]).then_inc(in_sem, 16)
            eng.dma_start(out=ones_sb[:], in_=ones_in[:]).then_inc(in_sem, 16)
            eng.dma_start(out=onesc_sb[:], in_=onesc_in[:]).then_inc(in_sem, 16)
            for k in (0, 2, 4, 6):
                eng.dma_start(
                    out=featsT_sb[:, k, :], in_=featsT_in[128 * k : 128 * (k + 1), :]
                ).then_inc(fsems[k], 16)

            eng.wait_ge(act_sem, 1)
            eng.dma_start(out=aff_out[:], in_=affT_sb[:]).then_inc(out_sem, 16)

            for it in range(ITERS - 1):
                eng.wait_ge(dve_sem, V_EX(it))
                eng.dma_start(out=cc_in[:], in_=cpart_sb[:]).then_inc(dma_sem, 16)
                eng.wait_ge(cc_sem, it + 1)
                src_ap = cc_out.ap().rearrange("(r e) o -> e (r o)", r=N_CORES)
                with nc.allow_non_contiguous_dma(reason="8x4B strided rank gather per partition"):
                    eng.dma_start(out=g8_sb[:], in_=src_ap).then_inc(dma_sem, 16)

            eng.wait_ge(act_sem, A_RT - 1)
            dst0 = rflat0_sb.ap()[0:1].rearrange("o (t p) -> o t p", p=128)
            eng.dma_start(out=dst0, in_=rt_sb[:, 0, :]).then_inc(dma_sem, 16)

            eng.wait_ge(dve_sem, V_SUB)
            eng.dma_start(out=zt_out[:], in_=zt_sb[:]).then_inc(out_sem, 16)
            eng.wait_ge(out_sem, 32)
            eng.wait_ge(dma_sem, 16 * D_RFLAT)

        # ---------------- tensor engine ----------------
        @block.tensor
        def _(eng):
            eng.wait_ge(in_sem, 16 * 12)
            for k in range(8):
                eng.wait_ge(fsems[k], 16)
                for n in range(4):
                    mm = eng.matmul(
                        ps_aff[:, 512 * n : 512 * (n + 1)],
                        centT_sb[:, k, :],
                        featsT_sb[:, k, 512 * n : 512 * (n + 1)],
                        start=(k == 0),
                        stop=(k == 7),
                    )
            mm.then_inc(pe_sem, 1)

            # iteration-0 R-step right after exp (E0_tm not needed for it)
            eng.wait_ge(act_sem, 2)
            for t in range(16):
                mm = eng.matmul(
                    ps_r[:, 2 * t : 2 * (t + 1)],
                    e0t_sb[:, 128 * t : 128 * (t + 1)],
                    v_sb[:],
                    start=True,
                    stop=True,
                )
            mm.then_inc(pe_sem, 1)

            for t in range(16):
                if t >= 2:
                    eng.wait_ge(act_sem, t + 1)  # copy t-2 freed this buffer
                buf = ps_tp if t % 2 == 0 else ps_tp2
                eng.transpose(
                    buf[:, 0:128], e0t_sb[:, 128 * t : 128 * (t + 1)], ident_sb[:]
                ).then_inc(pe_sem, 1)

            for it in range(ITERS):
                if it > 0:
                    eng.wait_ge(dve_sem, V_VU(it - 1))
                    for t in range(16):
                        mm = eng.matmul(
                            ps_r[:, 2 * t : 2 * (t + 1)],
                            e0t_sb[:, 128 * t : 128 * (t + 1)],
                            v_sb[:],
                            start=True,
                            stop=True,
                        )
                    mm.then_inc(pe_sem, 1)

                if it < ITERS - 1:
                    if it == 0:
                        eng.wait_ge(act_sem, A_E0TM)  # all e0tm copies landed
                    eng.wait_ge(dve_sem, V_W(it))
                    for t in range(16):
                        mm = eng.matmul(
                            ps_c[:, 0:2],
                            e0tm_sb[:, t, :],
                            w_sb[:, t, :],
                            start=(t == 0),
                            stop=(t == 15),
                        )
                    mm.then_inc(pe_sem, 1)

            eng.wait_ge(dve_sem, V_RL2)
            for s in range(2):
                eng.transpose(ps_tp[0:16, 0:128], rlog2_sb[:, :, s], ident_sb[:]).then_inc(pe_sem, 1)
                eng.wait_ge(act_sem, A_TEXP + 1 + s)  # ACT copied ps_tp before reuse

            eng.wait_ge(dma_sem, 16 * D_RFLAT)
            for s in range(2):
                rsrc = rflat0_sb if s == 0 else rflat1_sb
                for n in range(4):
                    mm = eng.matmul(
                        ps_aff[64 * s : 64 * (s + 1), 512 * n : 512 * (n + 1)],
                        ones_sb[0:1, :],
                        rsrc[0:1, 512 * n : 512 * (n + 1)],
                        start=True,
                        stop=True,
                    )
            mm.then_inc(pe_sem, 1)

        # ---------------- scalar (ACT) engine ----------------
        @block.scalar
        def _(eng):
            for k in (1, 3, 5, 7):
                eng.dma_start(
                    out=featsT_sb[:, k, :], in_=featsT_in[128 * k : 128 * (k + 1), :]
                ).then_inc(fsems[k], 16)
            eng.wait_ge(pe_sem, 1)
            eng.activation(affT_sb[:], ps_aff[:, 0:TOK], mybir.ActivationFunctionType.Copy).then_inc(act_sem, 1)
            eng.wait_ge(act_sem, 1)
            eng.activation(e0t_sb[:], affT_sb[:], mybir.ActivationFunctionType.Exp).then_inc(act_sem, 1)
            for t in range(16):
                eng.wait_ge(pe_sem, 3 + t)
                buf = ps_tp if t % 2 == 0 else ps_tp2
                eng.activation(
                    e0tm_sb[:, t, :], buf[:, 0:128], mybir.ActivationFunctionType.Copy
                ).then_inc(act_sem, 1)
            eng.wait_ge(pe_sem, P_LAST_R)
            eng.activation(
                rlog_sb.ap().rearrange("p t s -> p (t s)"),
                ps_r[:, 0:32],
                mybir.ActivationFunctionType.Ln,
            ).then_inc(act_sem, 1)
            eng.wait_ge(act_sem, A_LN)
            eng.activation(
                texp_sb[:],
                rlog_sb.ap().rearrange("p t s -> p (t s)"),
                mybir.ActivationFunctionType.Exp,
                scale=-1.0,
            ).then_inc(act_sem, 1)
            for s in range(2):
                eng.wait_ge(pe_sem, P_LAST_R + 1 + s)
                eng.activation(rt_sb[:, s, :], ps_tp[0:16, 0:128], mybir.ActivationFunctionType.Copy).then_inc(act_sem, 1)
            eng.wait_ge(act_sem, A_RT)
            dst1 = rflat1_sb.ap()[0:1].rearrange("o (t p) -> o t p", p=128)
            eng.dma_start(out=dst1, in_=rt_sb[:, 1, :]).then_inc(dma_sem, 16)


        # ---------------- vector (DVE) engine ----------------
        @block.vector
        def _(eng):
            for it in range(ITERS - 1):
                eng.wait_ge(pe_sem, P_R(it))
                eng.reciprocal(w_sb.ap().rearrange("p t s -> p (t s)"), ps_r[:, 0:32]).then_inc(dve_sem, 1)
                eng.wait_ge(pe_sem, P_C(it))
                eng.tensor_copy(cpart_sb[0:64, :], ps_c[0:64, 0:1])
                eng.tensor_copy(cpart_sb[64:128, :], ps_c[64:128, 1:2]).then_inc(dve_sem, 1)
                eng.wait_ge(dma_sem, 16 * (2 * it + 2))
                eng.tensor_reduce(
                    csum_sb[:], g8_sb[:], mybir.AxisListType.X, mybir.AluOpType.add
                ).then_inc(dve_sem, 1)
                eng.wait_ge(dve_sem, V_RD(it))
                eng.reciprocal(v_sb[0:64, 0:1], csum_sb[0:64, :])
                eng.reciprocal(v_sb[64:128, 1:2], csum_sb[64:128, :]).then_inc(dve_sem, 1)
            eng.wait_ge(pe_sem, P_LAST_R)
            eng.wait_ge(act_sem, A_TEXP)
            eng.tensor_mul(u_sb[:], ps_r[:, 0:32], texp_sb[:]).then_inc(dve_sem, 1)
            eng.wait_ge(dve_sem, V_U)
            eng.scalar_tensor_tensor(
                rlog2_sb.ap().rearrange("p t s -> p (t s)"),
                u_sb[:],
                1.0,
                rlog_sb.ap().rearrange("p t s -> p (t s)"),
                op0=mybir.AluOpType.subtract,
                op1=mybir.AluOpType.add,
            ).then_inc(dve_sem, 1)
            eng.wait_ge(pe_sem, P_ZB)
            eng.wait_ge(dve_sem, V_RL2)
            eng.tensor_sub(zt_sb[:], affT_sb[:], ps_aff[:, 0:TOK]).then_inc(dve_sem, 1)

        # ---------------- gpsimd: collectives ----------------
        @block.gpsimd
        def _(eng):
            for it in range(ITERS - 1):
                eng.wait_ge(dma_sem, 16 * (2 * it + 1))
                eng.collective_compute(
                    "AllGather",
                    mybir.AluOpType.bypass,
                    ins=[cc_in[:]],
                    outs=[cc_out[:]],
                    replica_groups=[core_ids],
                ).then_inc(cc_sem, 1)

    return nc


_CACHE = {}


def _get_nc():
    if "nc" not in _CACHE:
        _CACHE["nc"] = _build_nc()
    return _CACHE["nc"]


def make_in_maps(input_features, expert_centroids):
    feats = np.ascontiguousarray(np.asarray(input_features, dtype=np.float32).reshape(-1, D))
    cent = np.asarray(expert_centroids, dtype=np.float32).reshape(SE, D)

    featsT = np.ascontiguousarray(feats.T)
    centT = np.ascontiguousarray(cent.T)
    ident = np.eye(128, dtype=np.float32)
    ones = np.ones((1, 64), dtype=np.float32)
    onesc = np.ones((128, 1), dtype=np.float32)
    v0 = np.zeros((SE, 2), np.float32)
    v0[0:64, 0] = 1.0
    v0[64:128, 1] = 1.0

    in_maps = []
    for c in range(N_CORES):
        in_maps.append(
            {
                "featsT": np.ascontiguousarray(featsT[:, TOK * c : TOK * (c + 1)]),
                "centT": centT,
                "ident": ident,
                "ones": ones,
                "onesc": onesc,
                "v0": v0,
            }
        )
    return in_maps


def kernel(input_features: np.ndarray, expert_centroids: np.ndarray):
    in_maps = make_in_maps(input_features, expert_centroids)
    nc = _get_nc()
    res = run_bass_kernel_spmd(nc, in_maps, list(range(N_CORES)))

    zt = np.concatenate([res.results[c]["zt"] for c in range(N_CORES)], axis=1)
    afft = np.concatenate([res.results[c]["afft"] for c in range(N_CORES)], axis=1)

    Z = zt.reshape(KSLOT, E, N)
    A = afft.reshape(KSLOT, E, N)
    idx = np.empty((KSLOT, E, CAP), np.int32)
    vals = np.empty((KSLOT, E, CAP), np.float32)
    for k in range(KSLOT):
        for e in range(E):
            col = Z[k, e]
            part = np.sort(np.argpartition(-col, CAP - 1)[:CAP])
            order = part[np.argsort(-col[part], kind="stable")]
            idx[k, e] = order.astype(np.int32)
            vals[k, e] = A[k, e, order]
    return idx, vals



# revision 13
# speedup vs baseline: 1.1512x; 1.1512x over previous
"""Trainium2 Bass kernel for nn_BaseLayerGate (MoE balanced routing).

8 NeuronCores, data-parallel over tokens (2048/core).  Per core:
  - aff^T = centT.T @ featsT on PE (fp32), n-outer over 4 psum banks so the
    exp of each 512-col slab can start as soon as its bank closes.
  - Sinkhorn (10 iters) in reciprocal-potential form:
      R[n,s]  = sum_se E0[se,n] V[se,s]          (PE, 16 matvecs)
      C[se,s] = sum_n  E0^T[n,se] W[n,s]         (PE, accumulated)
    Global token-sum per iteration: AllGather of the [128,1] C-partial
    (9 serial exchanges; this is the latency floor of the problem).
    Cross-rank reduce of the gathered [8,128] block is ONE PE matvec.
  - Outputs: aff^T [128,2048] and the raw final R_sum [128,32].  The host
    computes r = ln(R_sum) exactly and the top-k ordering Z = aff - r
    (c-potential is a per-column shift and cannot change ordering).
"""

import numpy as np

import concourse.bass as bass
from concourse import mybir
from concourse.bass_utils import run_bass_kernel_spmd

N_CORES = 8
N = 16384
D = 1024
KSLOT = 2
E = 64
SE = KSLOT * E
CAP = N // E
TOK = N // N_CORES
ITERS = 10

F32 = mybir.dt.float32


def _build_nc():
    nc = bass.Bass()

    featsT_in = nc.declare_dram_parameter("featsT", [D, TOK], F32, isOutput=False)
    centT_in = nc.declare_dram_parameter("centT", [D, SE], F32, isOutput=False)
    v0_in = nc.declare_dram_parameter("v0", [SE, 2], F32, isOutput=False)
    ident_in = nc.declare_dram_parameter("ident", [128, 128], F32, isOutput=False)
    ones8_in = nc.declare_dram_parameter("ones8", [8, 1], F32, isOutput=False)

    aff_out = nc.declare_dram_parameter("afft", [SE, TOK], F32, isOutput=True)
    rsum_out = nc.declare_dram_parameter("rsum", [128, 32], F32, isOutput=True)

    cc_in = nc.dram_tensor("cc_in", [SE, 1], F32)
    cc_out = nc.dram_tensor("cc_out", [N_CORES * SE, 1], F32, addr_space="Shared")

    core_ids = list(range(N_CORES))

    from contextlib import ExitStack
    es = ExitStack()
    featsT_sb = es.enter_context(nc.sbuf_tensor("featsT_sb", [128, 8, TOK], F32))
    centT_sb = es.enter_context(nc.sbuf_tensor("centT_sb", [128, 8, SE], F32))
    e0t_sb = es.enter_context(nc.sbuf_tensor("e0t_sb", [128, TOK], F32))
    e0tm_sb = es.enter_context(nc.sbuf_tensor("e0tm_sb", [128, 16, 128], F32))
    affT_sb = es.enter_context(nc.sbuf_tensor("affT_sb", [128, TOK], F32))
    ident_sb = es.enter_context(nc.sbuf_tensor("ident_sb", [128, 128], F32))
    v_sb = es.enter_context(nc.sbuf_tensor("v_sb", [128, 2], F32))
    w_sb = es.enter_context(nc.sbuf_tensor("w_sb", [128, 16, 2], F32))
    cpart_sb = es.enter_context(nc.sbuf_tensor("cpart_sb", [128, 1], F32))
    g8p_sb = es.enter_context(nc.sbuf_tensor("g8p_sb", [8, 128], F32))
    ones8_sb = es.enter_context(nc.sbuf_tensor("ones8_sb", [8, 1], F32))
    rsum_sb = es.enter_context(nc.sbuf_tensor("rsum_sb", [128, 32], F32))
    scr_sb = es.enter_context(nc.sbuf_tensor("scr_sb", [128, 2], F32))

    ps_aff = es.enter_context(nc.psum_tensor("ps_aff", [128, 2048], F32))
    ps_tp = es.enter_context(nc.psum_tensor("ps_tp", [128, 512], F32))
    ps_tp2 = es.enter_context(nc.psum_tensor("ps_tp2", [128, 512], F32))
    ps_r = es.enter_context(nc.psum_tensor("ps_r", [128, 512], F32))
    ps_c = es.enter_context(nc.psum_tensor("ps_c", [128, 512], F32))

    block = es.enter_context(nc.Block())
    ident_sem = es.enter_context(nc.semaphore("ident_sem"))  # ident loaded
    in_sem = es.enter_context(nc.semaphore("in_sem"))      # centT loaded
    small_sem = es.enter_context(nc.semaphore("small_sem"))  # v0+ones8 (ACT queue)
    fsems = [es.enter_context(nc.semaphore(f"fsem{k}")) for k in range(8)]
    affn_sem = es.enter_context(nc.semaphore("affn_sem"))  # +1 per aff n-group
    exp_sem = es.enter_context(nc.semaphore("exp_sem"))    # +1 per exp slab
    tp_sem = es.enter_context(nc.semaphore("tp_sem"))      # +1 per transpose
    etm_sem = es.enter_context(nc.semaphore("etm_sem"))    # +1 per e0tm copy
    r_sem = es.enter_context(nc.semaphore("r_sem"))        # +1 per R-step
    w_sem = es.enter_context(nc.semaphore("w_sem"))        # +1 per W recip
    c_sem = es.enter_context(nc.semaphore("c_sem"))        # +1 per C-step
    cp_sem = es.enter_context(nc.semaphore("cp_sem"))      # +1 per cpart copy
    ccin_sem = es.enter_context(nc.semaphore("ccin_sem"))  # +16 per exch-out dma
    cc_sem = es.enter_context(nc.semaphore("cc_sem"))      # +1 per collective
    gath_sem = es.enter_context(nc.semaphore("gath_sem"))  # +16 per gather dma
    pcs_sem = es.enter_context(nc.semaphore("pcs_sem"))    # +1 per csum matvec
    v_sem = es.enter_context(nc.semaphore("v_sem"))        # +1 per v recip pair
    affc_sem = es.enter_context(nc.semaphore("affc_sem"))  # affT copied to sbuf
    rs_sem = es.enter_context(nc.semaphore("rs_sem"))      # rsum copied to sbuf
    out_sem = es.enter_context(nc.semaphore("out_sem"))

    with es:
        # ---------------- sync (SP): feats k0/k3/k6, exchange legs
        @block.sync
        def _(eng):
            for k in (0, 3, 6):
                eng.dma_start(
                    out=featsT_sb[:, k, :], in_=featsT_in[128 * k : 128 * (k + 1), :]
                ).then_inc(fsems[k], 16)

            for it in range(ITERS - 1):
                eng.wait_ge(cp_sem, it + 1)
                eng.dma_start(out=cc_in[:], in_=cpart_sb[:]).then_inc(ccin_sem, 16)
                eng.wait_ge(cc_sem, it + 1)
                gsrc = cc_out.ap().rearrange("(r e) o -> r (e o)", r=N_CORES)
                eng.dma_start(out=g8p_sb[:], in_=gsrc).then_inc(gath_sem, 16)

            eng.wait_ge(out_sem, 32)

        # ---------------- scalar (ACT): v0/ones8, feats k1/k4/k7, exps, outputs
        @block.scalar
        def _(eng):
            eng.dma_start(out=v_sb[:], in_=v0_in[:]).then_inc(small_sem, 16)
            eng.dma_start(out=ones8_sb[:], in_=ones8_in[:]).then_inc(small_sem, 16)
            for k in (1, 4, 7):
                eng.dma_start(
                    out=featsT_sb[:, k, :], in_=featsT_in[128 * k : 128 * (k + 1), :]
                ).then_inc(fsems[k], 16)
            # warm the Exp table while DMAs run
            eng.wait_ge(ident_sem, 16)
            eng.activation(scr_sb[:], ident_sb[:, 0:2], mybir.ActivationFunctionType.Exp)
            for n in range(4):
                eng.wait_ge(affn_sem, n + 1)
                eng.activation(
                    e0t_sb[:, 512 * n : 512 * (n + 1)],
                    ps_aff[:, 512 * n : 512 * (n + 1)],
                    mybir.ActivationFunctionType.Exp,
                ).then_inc(exp_sem, 1)
            # aff -> sbuf -> dram (during early iterations; ACT is idle)
            eng.activation(affT_sb[:], ps_aff[:, 0:TOK], mybir.ActivationFunctionType.Copy).then_inc(affc_sem, 1)
            eng.wait_ge(affc_sem, 1)
            eng.dma_start(out=aff_out[:], in_=affT_sb[:]).then_inc(out_sem, 16)
            # final R_sum out
            eng.wait_ge(r_sem, ITERS)
            eng.activation(rsum_sb[:], ps_r[:, 0:32], mybir.ActivationFunctionType.Copy).then_inc(rs_sem, 1)
            eng.wait_ge(rs_sem, 1)
            eng.dma_start(out=rsum_out[:], in_=rsum_sb[:]).then_inc(out_sem, 16)

        # ---------------- vector (DVE): e0tm copies, recips
        @block.vector
        def _(eng):
            for t in range(16):
                eng.wait_ge(tp_sem, t + 1)
                buf = ps_tp if t % 2 == 0 else ps_tp2
                eng.tensor_copy(e0tm_sb[:, t, :], buf[:, 0:128]).then_inc(etm_sem, 1)
            for it in range(ITERS - 1):
                eng.wait_ge(r_sem, it + 1)
                eng.reciprocal(w_sb.ap().rearrange("p t s -> p (t s)"), ps_r[:, 0:32]).then_inc(w_sem, 1)
                eng.wait_ge(c_sem, it + 1)
                eng.tensor_copy(cpart_sb[0:64, :], ps_c[0:64, 0:1])
                eng.tensor_copy(cpart_sb[64:128, :], ps_c[64:128, 1:2]).then_inc(cp_sem, 1)
                eng.wait_ge(pcs_sem, it + 1)
                eng.reciprocal(v_sb[0:64, 0:1], ps_tp[0:64, 0:1])
                eng.reciprocal(v_sb[64:128, 1:2], ps_tp[64:128, 0:1]).then_inc(v_sem, 1)

        # ---------------- gpsimd (Pool): ident/centT, feats k2/k5, collectives
        @block.gpsimd
        def _(eng):
            eng.dma_start(out=ident_sb[:], in_=ident_in[:]).then_inc(ident_sem, 16)
            csrc = centT_in.ap().rearrange("(k p) e -> p k e", p=128)
            with nc.allow_non_contiguous_dma(reason="8x512B strided centT load per partition"):
                eng.dma_start(out=centT_sb[:], in_=csrc).then_inc(in_sem, 16)
            for k in (2, 5):
                eng.dma_start(
                    out=featsT_sb[:, k, :], in_=featsT_in[128 * k : 128 * (k + 1), :]
                ).then_inc(fsems[k], 16)
            for it in range(ITERS - 1):
                eng.wait_ge(ccin_sem, 16 * (it + 1))
                eng.collective_compute(
                    "AllGather",
                    mybir.AluOpType.bypass,
                    ins=[cc_in[:]],
                    outs=[cc_out[:]],
                    replica_groups=[core_ids],
                ).then_inc(cc_sem, 1)

        # ---------------- tensor (PE) ----------------
        @block.tensor
        def _(eng):
            # p-state warmup on ident while feats stream in
            eng.wait_ge(ident_sem, 16)
            for _ in range(22):
                eng.transpose(ps_tp2[:, 0:128], ident_sb[:], ident_sb[:])
            eng.wait_ge(in_sem, 16)

            # aff matmul, n-outer so each 512-col psum bank closes early;
            # transposes of finished slabs interleave between n-groups.
            def tp_group(base):
                for t in range(base, base + 4):
                    if t >= 2:
                        eng.wait_ge(etm_sem, t - 1)
                    buf = ps_tp if t % 2 == 0 else ps_tp2
                    eng.transpose(
                        buf[:, 0:128], e0t_sb[:, 128 * t : 128 * (t + 1)], ident_sb[:]
                    ).then_inc(tp_sem, 1)

            for n in range(4):
                for k in range(8):
                    if n == 0:
                        eng.wait_ge(fsems[k], 16)
                    mm = eng.matmul(
                        ps_aff[:, 512 * n : 512 * (n + 1)],
                        centT_sb[:, k, :],
                        featsT_sb[:, k, 512 * n : 512 * (n + 1)],
                        start=(k == 0),
                        stop=(k == 7),
                    )
                mm.then_inc(affn_sem, 1)
                if n >= 2:
                    eng.wait_ge(exp_sem, n - 1)
                    tp_group(4 * (n - 2))
            eng.wait_ge(exp_sem, 3)
            tp_group(8)
            eng.wait_ge(exp_sem, 4)
            tp_group(12)

            eng.wait_ge(small_sem, 32)
            for it in range(ITERS):
                # R-step
                if it > 0:
                    eng.wait_ge(v_sem, it)
                for t in range(16):
                    mm = eng.matmul(
                        ps_r[:, 2 * t : 2 * (t + 1)],
                        e0t_sb[:, 128 * t : 128 * (t + 1)],
                        v_sb[:],
                        start=True,
                        stop=True,
                    )
                mm.then_inc(r_sem, 1)
                if it == ITERS - 1:
                    break
                # C-step
                if it == 0:
                    eng.wait_ge(etm_sem, 16)
                eng.wait_ge(w_sem, it + 1)
                for t in range(16):
                    mm = eng.matmul(
                        ps_c[:, 0:2],
                        e0tm_sb[:, t, :],
                        w_sb[:, t, :],
                        start=(t == 0),
                        stop=(t == 15),
                    )
                mm.then_inc(c_sem, 1)
                # cross-rank reduce of the gathered partials: one matvec
                eng.wait_ge(gath_sem, 16 * (it + 1))
                eng.matmul(
                    ps_tp[:, 0:1], g8p_sb[:], ones8_sb[:], start=True, stop=True
                ).then_inc(pcs_sem, 1)

    return nc


_CACHE = {}


def _get_nc():
    if "nc" not in _CACHE:
        _CACHE["nc"] = _build_nc()
    return _CACHE["nc"]


def make_in_maps(input_features, expert_centroids):
    feats = np.ascontiguousarray(np.asarray(input_features, dtype=np.float32).reshape(-1, D))
    cent = np.asarray(expert_centroids, dtype=np.float32).reshape(SE, D)

    featsT = np.ascontiguousarray(feats.T)
    centT = np.ascontiguousarray(cent.T)
    ident = np.eye(128, dtype=np.float32)
    ones8 = np.ones((8, 1), dtype=np.float32)
    v0 = np.zeros((SE, 2), np.float32)
    v0[0:64, 0] = 1.0
    v0[64:128, 1] = 1.0

    in_maps = []
    for c in range(N_CORES):
        in_maps.append(
            {
                "featsT": np.ascontiguousarray(featsT[:, TOK * c : TOK * (c + 1)]),
                "centT": centT,
                "ident": ident,
                "ones8": ones8,
                "v0": v0,
            }
        )
    return in_maps


def kernel(input_features: np.ndarray, expert_centroids: np.ndarray):
    in_maps = make_in_maps(input_features, expert_centroids)
    nc = _get_nc()
    res = run_bass_kernel_spmd(nc, in_maps, list(range(N_CORES)))

    afft = np.concatenate([res.results[c]["afft"] for c in range(N_CORES)], axis=1)
    # rsum[c][p, 2t+s] = R_sum for local token 128t+p, slot s
    r = np.empty((KSLOT, N), np.float64)
    for c in range(N_CORES):
        rs = np.asarray(res.results[c]["rsum"], dtype=np.float64)  # [128, 32]
        for s in range(KSLOT):
            blk = rs[:, s::2]  # [128 p, 16 t]
            r[s, TOK * c : TOK * (c + 1)] = blk.T.reshape(-1)  # token = 128t+p

    A = afft.reshape(KSLOT, E, N)
    Z = A - np.log(r)[:, None, :]
    idx = np.empty((KSLOT, E, CAP), np.int32)
    vals = np.empty((KSLOT, E, CAP), np.float32)
    for k in range(KSLOT):
        for e in range(E):
            col = Z[k, e]
            part = np.sort(np.argpartition(-col, CAP - 1)[:CAP])
            order = part[np.argsort(-col[part], kind="stable")]
            idx[k, e] = order.astype(np.int32)
            vals[k, e] = A[k, e, order]
    return idx, vals


# revision 18
# speedup vs baseline: 1.2023x; 1.0443x over previous
"""Trainium2 Bass kernel for nn_BaseLayerGate (MoE balanced routing).

8 NeuronCores, data-parallel over tokens (2048/core).  Per core:
  - aff^T = centT.T @ featsT on PE in split-bf16 (fp32 = hi + lo bf16;
    3-term product, fl*cl dropped -> ~4e-6 rel err, 1 cyc/row vs 4 for
    fp32).  n-outer over 4 psum banks; the first 512-col slice of every
    feat chunk is DMA'd separately so bank 0 can start early.
  - Sinkhorn in reciprocal-potential form, 9 on-chip half-iterations:
      R[n,s]  = sum_se E0[se,n] V[se,s]        (PE, 16 matvecs)
      C[se,s] = sum_n  E0^T[n,se] W[n,s]       (PE, accumulated)
    One AllGather of the [128,1] C-partial per iteration (9 serial
    exchanges - the latency floor).  Cross-rank reduce is one PE matvec.
  - The 10th R-step runs on the HOST: the kernel ships the last gathered
    partials (gath8) + aff; the host computes V8 = 1/sum(partials),
    r = ln(exp(aff) @ V8), and the top-k ordering Z = aff - r
    (the c-potential is a per-column shift and cannot change ordering).
"""

import numpy as np
import ml_dtypes

import concourse.bass as bass
from concourse import mybir
from concourse.bass_utils import run_bass_kernel_spmd

N_CORES = 8
N = 16384
D = 1024
KSLOT = 2
E = 64
SE = KSLOT * E
CAP = N // E
TOK = N // N_CORES
ITERS = 10
EXCH = ITERS - 1          # 9 collectives
NSLICE = 512              # early slice columns

F32 = mybir.dt.float32
BF16 = mybir.dt.bfloat16


def _build_nc():
    nc = bass.Bass()

    fh_in = nc.declare_dram_parameter("fh", [D, TOK], BF16, isOutput=False)
    fl_in = nc.declare_dram_parameter("fl", [D, TOK], BF16, isOutput=False)
    ch_in = nc.declare_dram_parameter("ch", [D, SE], BF16, isOutput=False)
    cl_in = nc.declare_dram_parameter("cl", [D, SE], BF16, isOutput=False)
    v0_in = nc.declare_dram_parameter("v0", [SE, 2], F32, isOutput=False)
    ident_in = nc.declare_dram_parameter("ident", [128, 128], F32, isOutput=False)
    ones8_in = nc.declare_dram_parameter("ones8", [8, 1], F32, isOutput=False)

    aff_out = nc.declare_dram_parameter("afft", [SE, TOK], F32, isOutput=True)
    g8_out = nc.declare_dram_parameter("gath8", [N_CORES * SE, 1], F32, isOutput=True)

    cc_in = nc.dram_tensor("cc_in", [SE, 1], F32)
    cc_out = nc.dram_tensor("cc_out", [N_CORES * SE, 1], F32, addr_space="Shared")

    core_ids = list(range(N_CORES))

    from contextlib import ExitStack
    es = ExitStack()
    fh_sb = es.enter_context(nc.sbuf_tensor("fh_sb", [128, 8, TOK], BF16))
    fl_sb = es.enter_context(nc.sbuf_tensor("fl_sb", [128, 8, TOK], BF16))
    ch_sb = es.enter_context(nc.sbuf_tensor("ch_sb", [128, 8, SE], BF16))
    cl_sb = es.enter_context(nc.sbuf_tensor("cl_sb", [128, 8, SE], BF16))
    e0t_sb = es.enter_context(nc.sbuf_tensor("e0t_sb", [128, TOK], F32))
    e0tm_sb = es.enter_context(nc.sbuf_tensor("e0tm_sb", [128, 16, 128], F32))
    affT_sb = es.enter_context(nc.sbuf_tensor("affT_sb", [128, TOK], F32))
    ident_sb = es.enter_context(nc.sbuf_tensor("ident_sb", [128, 128], F32))
    v_sb = es.enter_context(nc.sbuf_tensor("v_sb", [128, 2], F32))
    w_sb = es.enter_context(nc.sbuf_tensor("w_sb", [128, 16, 2], F32))
    cpart_sb = es.enter_context(nc.sbuf_tensor("cpart_sb", [128, 1], F32))
    g8p_sb = es.enter_context(nc.sbuf_tensor("g8p_sb", [8, 128], F32))
    ones8_sb = es.enter_context(nc.sbuf_tensor("ones8_sb", [8, 1], F32))
    scr_sb = es.enter_context(nc.sbuf_tensor("scr_sb", [128, 2], F32))

    ps_aff = es.enter_context(nc.psum_tensor("ps_aff", [128, 2048], F32))
    ps_tp = es.enter_context(nc.psum_tensor("ps_tp", [128, 512], F32))
    ps_tp2 = es.enter_context(nc.psum_tensor("ps_tp2", [128, 512], F32))
    ps_r = es.enter_context(nc.psum_tensor("ps_r", [128, 512], F32))
    ps_c = es.enter_context(nc.psum_tensor("ps_c", [128, 512], F32))

    block = es.enter_context(nc.Block())
    ident_sem = es.enter_context(nc.semaphore("ident_sem"))
    ch_sem = es.enter_context(nc.semaphore("ch_sem"))
    cl_sem = es.enter_context(nc.semaphore("cl_sem"))
    small_sem = es.enter_context(nc.semaphore("small_sem"))  # v0 + ones8
    # per-piece feat sems: [hi/lo][k][slice/rest]
    fsl = [[es.enter_context(nc.semaphore(f"fsl{p}_{k}")) for k in range(8)] for p in range(2)]
    frs = [[es.enter_context(nc.semaphore(f"frs{p}_{k}")) for k in range(8)] for p in range(2)]
    affn_sem = es.enter_context(nc.semaphore("affn_sem"))
    exp_sem = es.enter_context(nc.semaphore("exp_sem"))
    tp_sem = es.enter_context(nc.semaphore("tp_sem"))
    etm_sem = es.enter_context(nc.semaphore("etm_sem"))
    r_sem = es.enter_context(nc.semaphore("r_sem"))
    w_sem = es.enter_context(nc.semaphore("w_sem"))
    c_sem = es.enter_context(nc.semaphore("c_sem"))
    cp_sem = es.enter_context(nc.semaphore("cp_sem"))
    ccin_sem = es.enter_context(nc.semaphore("ccin_sem"))
    cc_sem = es.enter_context(nc.semaphore("cc_sem"))
    gath_sem = es.enter_context(nc.semaphore("gath_sem"))
    pcs_sem = es.enter_context(nc.semaphore("pcs_sem"))
    v_sem = es.enter_context(nc.semaphore("v_sem"))
    affc_sem = es.enter_context(nc.semaphore("affc_sem"))
    out_sem = es.enter_context(nc.semaphore("out_sem"))

    fparts = (fh_in, fl_in)
    fsbs = (fh_sb, fl_sb)

    def feat_slice_dma(eng, p, k):
        eng.dma_start(
            out=fsbs[p][:, k, 0:NSLICE],
            in_=fparts[p][128 * k : 128 * (k + 1), 0:NSLICE],
        ).then_inc(fsl[p][k], 16)

    def feat_rest_dma(eng, p, k):
        eng.dma_start(
            out=fsbs[p][:, k, NSLICE:TOK],
            in_=fparts[p][128 * k : 128 * (k + 1), NSLICE:TOK],
        ).then_inc(frs[p][k], 16)

    with es:
        # ---------------- sync (SP): cent-hi, feat pieces, exchange legs, final out
        @block.sync
        def _(eng):
            chsrc = ch_in.ap().rearrange("(k p) e -> p k e", p=128)
            with nc.allow_non_contiguous_dma(reason="strided cent load per partition"):
                eng.dma_start(out=ch_sb[:], in_=chsrc).then_inc(ch_sem, 16)
            for p, k in ((0, 0), (1, 1), (0, 3), (1, 4), (0, 6), (1, 7)):
                feat_slice_dma(eng, p, k)
            for p, k in ((0, 0), (1, 1), (0, 3), (1, 4), (0, 6), (1, 7)):
                feat_rest_dma(eng, p, k)

            for it in range(EXCH):
                eng.wait_ge(cp_sem, it + 1)
                eng.dma_start(out=cc_in[:], in_=cpart_sb[:]).then_inc(ccin_sem, 16)
                if it < EXCH - 1:
                    eng.wait_ge(cc_sem, it + 1)
                    gsrc = cc_out.ap().rearrange("(r e) o -> r (e o)", r=N_CORES)
                    eng.dma_start(out=g8p_sb[:], in_=gsrc).then_inc(gath_sem, 16)
            eng.wait_ge(cc_sem, EXCH)
            eng.dma_start(out=g8_out[:], in_=cc_out[:]).then_inc(out_sem, 16)
            eng.wait_ge(out_sem, 32)

        # ---------------- scalar (ACT): cent-lo, feat pieces, v0/ones8, exps, aff out
        @block.scalar
        def _(eng):
            clsrc = cl_in.ap().rearrange("(k p) e -> p k e", p=128)
            with nc.allow_non_contiguous_dma(reason="strided cent load per partition"):
                eng.dma_start(out=cl_sb[:], in_=clsrc).then_inc(cl_sem, 16)
            for p, k in ((0, 1), (1, 2), (0, 4), (1, 5), (0, 7)):
                feat_slice_dma(eng, p, k)
            eng.dma_start(out=v_sb[:], in_=v0_in[:]).then_inc(small_sem, 16)
            eng.dma_start(out=ones8_sb[:], in_=ones8_in[:]).then_inc(small_sem, 16)
            for p, k in ((0, 1), (1, 2), (0, 4), (1, 5), (0, 7)):
                feat_rest_dma(eng, p, k)
            # warm the Exp table while DMAs run
            eng.wait_ge(ident_sem, 16)
            eng.activation(scr_sb[:], ident_sb[:, 0:2], mybir.ActivationFunctionType.Exp)
            for n in range(4):
                eng.wait_ge(affn_sem, n + 1)
                eng.activation(
                    e0t_sb[:, 512 * n : 512 * (n + 1)],
                    ps_aff[:, 512 * n : 512 * (n + 1)],
                    mybir.ActivationFunctionType.Exp,
                ).then_inc(exp_sem, 1)
            eng.activation(affT_sb[:], ps_aff[:, 0:TOK], mybir.ActivationFunctionType.Copy).then_inc(affc_sem, 1)
            eng.wait_ge(affc_sem, 1)
            eng.dma_start(out=aff_out[:], in_=affT_sb[:]).then_inc(out_sem, 16)

        # ---------------- vector (DVE): e0tm copies, recips
        @block.vector
        def _(eng):
            for t in range(16):
                eng.wait_ge(tp_sem, t + 1)
                buf = ps_tp if t % 2 == 0 else ps_tp2
                eng.tensor_copy(e0tm_sb[:, t, :], buf[:, 0:128]).then_inc(etm_sem, 1)
            for it in range(EXCH):
                eng.wait_ge(r_sem, it + 1)
                eng.reciprocal(w_sb.ap().rearrange("p t s -> p (t s)"), ps_r[:, 0:32]).then_inc(w_sem, 1)
                eng.wait_ge(c_sem, it + 1)
                eng.tensor_copy(cpart_sb[0:64, :], ps_c[0:64, 0:1])
                eng.tensor_copy(cpart_sb[64:128, :], ps_c[64:128, 1:2]).then_inc(cp_sem, 1)
                if it < EXCH - 1:
                    eng.wait_ge(pcs_sem, it + 1)
                    eng.reciprocal(v_sb[0:64, 0:1], ps_tp[0:64, 0:1])
                    eng.reciprocal(v_sb[64:128, 1:2], ps_tp[64:128, 0:1]).then_inc(v_sem, 1)

        # ---------------- gpsimd (Pool): ident, feat pieces, collectives
        @block.gpsimd
        def _(eng):
            eng.dma_start(out=ident_sb[:], in_=ident_in[:]).then_inc(ident_sem, 16)
            for p, k in ((1, 0), (0, 2), (1, 3), (0, 5), (1, 6)):
                feat_slice_dma(eng, p, k)
            for p, k in ((1, 0), (0, 2), (1, 3), (0, 5), (1, 6)):
                feat_rest_dma(eng, p, k)

            for it in range(EXCH):
                eng.wait_ge(ccin_sem, 16 * (it + 1))
                eng.collective_compute(
                    "AllGather",
                    mybir.AluOpType.bypass,
                    ins=[cc_in[:]],
                    outs=[cc_out[:]],
                    replica_groups=[core_ids],
                ).then_inc(cc_sem, 1)

        # ---------------- tensor (PE) ----------------
        @block.tensor
        def _(eng):
            # p-state warmup on ident while feats stream in
            eng.wait_ge(ident_sem, 16)
            for _ in range(22):
                eng.transpose(ps_tp2[:, 0:128], ident_sb[:], ident_sb[:])
            eng.wait_ge(ch_sem, 16)
            eng.wait_ge(cl_sem, 16)

            def tp_group(base):
                for t in range(base, base + 4):
                    if t >= 2:
                        eng.wait_ge(etm_sem, t - 1)
                    buf = ps_tp if t % 2 == 0 else ps_tp2
                    eng.transpose(
                        buf[:, 0:128], e0t_sb[:, 128 * t : 128 * (t + 1)], ident_sb[:]
                    ).then_inc(tp_sem, 1)

            # 3-term split-bf16 aff matmul, n-outer
            for n in range(4):
                lo, hi = 512 * n, 512 * (n + 1)
                first = True
                for k in range(8):
                    if n == 0:
                        eng.wait_ge(fsl[0][k], 16)
                        eng.wait_ge(fsl[1][k], 16)
                    elif n == 1:
                        eng.wait_ge(frs[0][k], 16)
                        eng.wait_ge(frs[1][k], 16)
                    for cw, fm in ((ch_sb, fh_sb), (cl_sb, fh_sb), (ch_sb, fl_sb)):
                        mm = eng.matmul(
                            ps_aff[:, lo:hi],
                            cw[:, k, :],
                            fm[:, k, lo:hi],
                            start=first,
                            stop=(k == 7 and fm is fl_sb),
                        )
                        first = False
                mm.then_inc(affn_sem, 1)
                if n >= 1:
                    eng.wait_ge(exp_sem, n)
                    tp_group(4 * (n - 1))
            # iteration-0 R-step chunks 0..11 while exp3 lands
            eng.wait_ge(small_sem, 32)
            for t in range(12):
                eng.matmul(
                    ps_r[:, 2 * t : 2 * (t + 1)],
                    e0t_sb[:, 128 * t : 128 * (t + 1)],
                    v_sb[:],
                    start=True,
                    stop=True,
                )
            eng.wait_ge(exp_sem, 4)
            tp_group(12)
            for t in range(12, 16):
                mm = eng.matmul(
                    ps_r[:, 2 * t : 2 * (t + 1)],
                    e0t_sb[:, 128 * t : 128 * (t + 1)],
                    v_sb[:],
                    start=True,
                    stop=True,
                )
            mm.then_inc(r_sem, 1)

            for it in range(EXCH):
                # C-step
                if it == 0:
                    eng.wait_ge(etm_sem, 16)
                eng.wait_ge(w_sem, it + 1)
                for t in range(16):
                    mm = eng.matmul(
                        ps_c[:, 0:2],
                        e0tm_sb[:, t, :],
                        w_sb[:, t, :],
                        start=(t == 0),
                        stop=(t == 15),
                    )
                mm.then_inc(c_sem, 1)
                if it == EXCH - 1:
                    break
                # cross-rank reduce of gathered partials: one matvec
                eng.wait_ge(gath_sem, 16 * (it + 1))
                eng.matmul(
                    ps_tp[:, 0:1], g8p_sb[:], ones8_sb[:], start=True, stop=True
                ).then_inc(pcs_sem, 1)
                # next R-step
                eng.wait_ge(v_sem, it + 1)
                for t in range(16):
                    mm = eng.matmul(
                        ps_r[:, 2 * t : 2 * (t + 1)],
                        e0t_sb[:, 128 * t : 128 * (t + 1)],
                        v_sb[:],
                        start=True,
                        stop=True,
                    )
                mm.then_inc(r_sem, 1)

    return nc


_CACHE = {}


def _get_nc():
    if "nc" not in _CACHE:
        _CACHE["nc"] = _build_nc()
    return _CACHE["nc"]


def make_in_maps(input_features, expert_centroids):
    feats = np.ascontiguousarray(np.asarray(input_features, dtype=np.float32).reshape(-1, D))
    cent = np.asarray(expert_centroids, dtype=np.float32).reshape(SE, D)

    featsT = np.ascontiguousarray(feats.T)          # [D, N]
    centT = np.ascontiguousarray(cent.T)            # [D, SE]
    fT_h = featsT.astype(ml_dtypes.bfloat16)
    fT_l = (featsT - fT_h.astype(np.float32)).astype(ml_dtypes.bfloat16)
    cT_h = centT.astype(ml_dtypes.bfloat16)
    cT_l = (centT - cT_h.astype(np.float32)).astype(ml_dtypes.bfloat16)

    ident = np.eye(128, dtype=np.float32)
    ones8 = np.ones((8, 1), dtype=np.float32)
    v0 = np.zeros((SE, 2), np.float32)
    v0[0:64, 0] = 1.0
    v0[64:128, 1] = 1.0

    in_maps = []
    for c in range(N_CORES):
        sl = slice(TOK * c, TOK * (c + 1))
        in_maps.append(
            {
                "fh": np.ascontiguousarray(fT_h[:, sl]),
                "fl": np.ascontiguousarray(fT_l[:, sl]),
                "ch": cT_h,
                "cl": cT_l,
                "ident": ident,
                "ones8": ones8,
                "v0": v0,
            }
        )
    return in_maps


def kernel(input_features: np.ndarray, expert_centroids: np.ndarray):
    in_maps = make_in_maps(input_features, expert_centroids)
    nc = _get_nc()
    res = run_bass_kernel_spmd(nc, in_maps, list(range(N_CORES)))

    afft = np.concatenate([res.results[c]["afft"] for c in range(N_CORES)], axis=1)
    # final exchange partials are identical on every core; host does the
    # 10th R-step: V8 = 1/sum_r partials, r = ln(exp(aff) @ V8)
    g8 = np.asarray(res.results[0]["gath8"], dtype=np.float32).reshape(N_CORES, SE)
    csum = g8.sum(axis=0)
    v8 = (1.0 / csum).astype(np.float32)  # [SE]

    A = afft.reshape(KSLOT, E, N)                      # [k, e, n]
    E0 = np.exp(A.astype(np.float32))
    r = np.empty((KSLOT, N), np.float64)
    for k in range(KSLOT):
        r[k] = np.log(np.einsum('en,e->n', E0[k], v8[64 * k : 64 * (k + 1)], dtype=np.float64))
    Z = A - r[:, None, :]
    idx = np.empty((KSLOT, E, CAP), np.int32)
    vals = np.empty((KSLOT, E, CAP), np.float32)
    for k in range(KSLOT):
        for e in range(E):
            col = Z[k, e]
            part = np.sort(np.argpartition(-col, CAP - 1)[:CAP])
            order = part[np.argsort(-col[part], kind="stable")]
            idx[k, e] = order.astype(np.int32)
            vals[k, e] = A[k, e, order]
    return idx, vals


# revision 25
# speedup vs baseline: 1.2051x; 1.0023x over previous
"""Trainium2 Bass kernel for nn_BaseLayerGate (MoE balanced routing).

8 NeuronCores, data-parallel over tokens (2048/core).  Per core:
  - aff^T = centT.T @ featsT on PE in split-bf16 (fp32 = hi + lo bf16;
    3-term product, fl*cl dropped -> ~4e-6 rel err, 1 cyc/row vs 4 for
    fp32).  n-outer over 4 psum banks; the first 512-col slice of every
    feat chunk is DMA'd separately so bank 0 can start early.
  - Sinkhorn in reciprocal-potential form, 9 on-chip half-iterations:
      R[n,s]  = sum_se E0[se,n] V[se,s]        (PE, 16 matvecs)
      C[se,s] = sum_n  E0^T[n,se] W[n,s]       (PE, accumulated)
    One AllGather of the [128,1] C-partial per iteration (9 serial
    exchanges - the latency floor).  Cross-rank reduce is one PE matvec.
  - The 10th R-step runs on the HOST: the kernel ships the last gathered
    partials (gath8) + aff; the host computes V8 = 1/sum(partials),
    r = ln(exp(aff) @ V8), and the top-k ordering Z = aff - r
    (the c-potential is a per-column shift and cannot change ordering).
"""

import numpy as np
import ml_dtypes

import concourse.bass as bass
from concourse import mybir
from concourse.bass_utils import run_bass_kernel_spmd

N_CORES = 8
N = 16384
D = 1024
KSLOT = 2
E = 64
SE = KSLOT * E
CAP = N // E
TOK = N // N_CORES
ITERS = 10
EXCH = ITERS - 1          # 9 collectives
NSLICE = 512              # early slice columns

F32 = mybir.dt.float32
BF16 = mybir.dt.bfloat16


def _build_nc():
    nc = bass.Bass()

    fh_in = nc.declare_dram_parameter("fh", [D, TOK], BF16, isOutput=False)
    fl_in = nc.declare_dram_parameter("fl", [D, TOK], BF16, isOutput=False)
    ch_in = nc.declare_dram_parameter("ch", [D, SE], BF16, isOutput=False)
    cl_in = nc.declare_dram_parameter("cl", [D, SE], BF16, isOutput=False)
    v0_in = nc.declare_dram_parameter("v0", [SE, 2], F32, isOutput=False)
    ident_in = nc.declare_dram_parameter("ident", [128, 128], F32, isOutput=False)
    ones8_in = nc.declare_dram_parameter("ones8", [8, 1], F32, isOutput=False)

    aff_out = nc.declare_dram_parameter("afft", [SE, TOK], F32, isOutput=True)
    g8_out = nc.declare_dram_parameter("gath8", [N_CORES * SE, 1], F32, isOutput=True)

    cc_in = nc.dram_tensor("cc_in", [SE, 1], F32)
    cc_out = nc.dram_tensor("cc_out", [N_CORES * SE, 1], F32, addr_space="Shared")

    core_ids = list(range(N_CORES))

    from contextlib import ExitStack
    es = ExitStack()
    fh_sb = es.enter_context(nc.sbuf_tensor("fh_sb", [128, 8, TOK], BF16))
    fl_sb = es.enter_context(nc.sbuf_tensor("fl_sb", [128, 8, TOK], BF16))
    ch_sb = es.enter_context(nc.sbuf_tensor("ch_sb", [128, 8, SE], BF16))
    cl_sb = es.enter_context(nc.sbuf_tensor("cl_sb", [128, 8, SE], BF16))
    e0t_sb = es.enter_context(nc.sbuf_tensor("e0t_sb", [128, TOK], F32))
    e0tm_sb = es.enter_context(nc.sbuf_tensor("e0tm_sb", [128, 16, 128], F32))
    affT_sb = es.enter_context(nc.sbuf_tensor("affT_sb", [128, TOK], F32))
    ident_sb = es.enter_context(nc.sbuf_tensor("ident_sb", [128, 128], F32))
    v_sb = es.enter_context(nc.sbuf_tensor("v_sb", [128, 2], F32))
    w_sb = es.enter_context(nc.sbuf_tensor("w_sb", [128, 16, 2], F32))
    cpart_sb = es.enter_context(nc.sbuf_tensor("cpart_sb", [128, 1], F32))
    g8p_sb = es.enter_context(nc.sbuf_tensor("g8p_sb", [8, 128], F32))
    ones8_sb = es.enter_context(nc.sbuf_tensor("ones8_sb", [8, 1], F32))
    scr_sb = es.enter_context(nc.sbuf_tensor("scr_sb", [128, 2], F32))

    ps_aff = es.enter_context(nc.psum_tensor("ps_aff", [128, 2048], F32))
    ps_tp = es.enter_context(nc.psum_tensor("ps_tp", [128, 512], F32))
    ps_tp2 = es.enter_context(nc.psum_tensor("ps_tp2", [128, 512], F32))
    ps_r = es.enter_context(nc.psum_tensor("ps_r", [128, 512], F32))
    ps_c = es.enter_context(nc.psum_tensor("ps_c", [128, 512], F32))

    block = es.enter_context(nc.Block())
    ident_sem = es.enter_context(nc.semaphore("ident_sem"))
    ch_sem = es.enter_context(nc.semaphore("ch_sem"))
    cl_sem = es.enter_context(nc.semaphore("cl_sem"))
    small_sem = es.enter_context(nc.semaphore("small_sem"))  # v0 + ones8
    # per-piece feat sems: [hi/lo][k][slice/rest]
    fsl = [[es.enter_context(nc.semaphore(f"fsl{p}_{k}")) for k in range(8)] for p in range(2)]
    frs = [[es.enter_context(nc.semaphore(f"frs{p}_{k}")) for k in range(8)] for p in range(2)]
    affn_sem = es.enter_context(nc.semaphore("affn_sem"))
    exp_sem = es.enter_context(nc.semaphore("exp_sem"))
    tp_sem = es.enter_context(nc.semaphore("tp_sem"))
    etm_sem = es.enter_context(nc.semaphore("etm_sem"))
    r_sem = es.enter_context(nc.semaphore("r_sem"))
    w_sem = es.enter_context(nc.semaphore("w_sem"))
    c_sem = es.enter_context(nc.semaphore("c_sem"))
    cp_sem = es.enter_context(nc.semaphore("cp_sem"))
    ccin_sem = es.enter_context(nc.semaphore("ccin_sem"))
    cc_sem = es.enter_context(nc.semaphore("cc_sem"))
    gath_sem = es.enter_context(nc.semaphore("gath_sem"))
    pcs_sem = es.enter_context(nc.semaphore("pcs_sem"))
    v_sem = es.enter_context(nc.semaphore("v_sem"))
    affc_sem = es.enter_context(nc.semaphore("affc_sem"))
    out_sem = es.enter_context(nc.semaphore("out_sem"))

    fparts = (fh_in, fl_in)
    fsbs = (fh_sb, fl_sb)

    def feat_slice_dma(eng, p, k):
        eng.dma_start(
            out=fsbs[p][:, k, 0:NSLICE],
            in_=fparts[p][128 * k : 128 * (k + 1), 0:NSLICE],
        ).then_inc(fsl[p][k], 16)

    def feat_rest_dma(eng, p, k):
        eng.dma_start(
            out=fsbs[p][:, k, NSLICE:TOK],
            in_=fparts[p][128 * k : 128 * (k + 1), NSLICE:TOK],
        ).then_inc(frs[p][k], 16)

    with es:
        # ---------------- sync (SP): cent-hi, feat pieces, exchange legs, final out
        @block.sync
        def _(eng):
            chsrc = ch_in.ap().rearrange("(k p) e -> p k e", p=128)
            with nc.allow_non_contiguous_dma(reason="strided cent load per partition"):
                eng.dma_start(out=ch_sb[:], in_=chsrc).then_inc(ch_sem, 16)
            for p, k in ((0, 0), (1, 1), (0, 3), (1, 4), (0, 6), (1, 7)):
                feat_slice_dma(eng, p, k)
            for p, k in ((0, 0), (1, 1), (0, 3), (1, 4), (0, 6), (1, 7)):
                feat_rest_dma(eng, p, k)

            for it in range(EXCH):
                eng.wait_ge(cp_sem, it + 1)
                eng.dma_start(out=cc_in[:], in_=cpart_sb[:]).then_inc(ccin_sem, 16)
                if it < EXCH - 1:
                    eng.wait_ge(cc_sem, it + 1)
                    gsrc = cc_out.ap().rearrange("(r e) o -> r (e o)", r=N_CORES)
                    eng.dma_start(out=g8p_sb[:], in_=gsrc).then_inc(gath_sem, 16)
            eng.wait_ge(cc_sem, EXCH)
            eng.dma_start(out=g8_out[:], in_=cc_out[:]).then_inc(out_sem, 16)
            eng.wait_ge(out_sem, 32)

        # ---------------- scalar (ACT): cent-lo, feat pieces, v0/ones8, exps, aff out
        @block.scalar
        def _(eng):
            clsrc = cl_in.ap().rearrange("(k p) e -> p k e", p=128)
            with nc.allow_non_contiguous_dma(reason="strided cent load per partition"):
                eng.dma_start(out=cl_sb[:], in_=clsrc).then_inc(cl_sem, 16)
            for p, k in ((0, 1), (1, 2), (0, 4), (1, 5), (0, 7)):
                feat_slice_dma(eng, p, k)
            eng.dma_start(out=v_sb[:], in_=v0_in[:]).then_inc(small_sem, 16)
            eng.dma_start(out=ones8_sb[:], in_=ones8_in[:]).then_inc(small_sem, 16)
            for p, k in ((0, 1), (1, 2), (0, 4), (1, 5), (0, 7)):
                feat_rest_dma(eng, p, k)
            # warm the Exp table while DMAs run
            eng.wait_ge(ident_sem, 16)
            eng.activation(scr_sb[:], ident_sb[:, 0:2], mybir.ActivationFunctionType.Exp)
            for n in range(4):
                eng.wait_ge(affn_sem, n + 1)
                eng.activation(
                    e0t_sb[:, 512 * n : 512 * (n + 1)],
                    ps_aff[:, 512 * n : 512 * (n + 1)],
                    mybir.ActivationFunctionType.Exp,
                ).then_inc(exp_sem, 1)
            eng.activation(affT_sb[:], ps_aff[:, 0:TOK], mybir.ActivationFunctionType.Copy).then_inc(affc_sem, 1)
            eng.wait_ge(affc_sem, 1)
            eng.dma_start(out=aff_out[:], in_=affT_sb[:]).then_inc(out_sem, 16)

        # ---------------- vector (DVE): e0tm copies, recips
        @block.vector
        def _(eng):
            for t in range(16):
                eng.wait_ge(tp_sem, t + 1)
                buf = (ps_tp, ps_tp2, ps_c)[t % 3]
                eng.tensor_copy(e0tm_sb[:, t, :], buf[:, 0:128]).then_inc(etm_sem, 1)
            for it in range(EXCH):
                eng.wait_ge(r_sem, it + 1)
                eng.reciprocal(w_sb.ap().rearrange("p t s -> p (t s)"), ps_r[:, 0:32]).then_inc(w_sem, 1)
                eng.wait_ge(c_sem, it + 1)
                eng.tensor_copy(cpart_sb[0:64, :], ps_c[0:64, 0:1])
                eng.tensor_copy(cpart_sb[64:128, :], ps_c[64:128, 1:2]).then_inc(cp_sem, 1)
                if it < EXCH - 1:
                    eng.wait_ge(pcs_sem, it + 1)
                    eng.reciprocal(v_sb[0:64, 0:1], ps_tp[0:64, 0:1])
                    eng.reciprocal(v_sb[64:128, 1:2], ps_tp[64:128, 0:1]).then_inc(v_sem, 1)

        # ---------------- gpsimd (Pool): ident, feat pieces, collectives
        @block.gpsimd
        def _(eng):
            eng.dma_start(out=ident_sb[:], in_=ident_in[:]).then_inc(ident_sem, 16)
            for p, k in ((1, 0), (0, 2), (1, 3), (0, 5), (1, 6)):
                feat_slice_dma(eng, p, k)
            for p, k in ((1, 0), (0, 2), (1, 3), (0, 5), (1, 6)):
                feat_rest_dma(eng, p, k)

            for it in range(EXCH):
                eng.wait_ge(ccin_sem, 16 * (it + 1))
                eng.collective_compute(
                    "AllGather",
                    mybir.AluOpType.bypass,
                    ins=[cc_in[:]],
                    outs=[cc_out[:]],
                    replica_groups=[core_ids],
                ).then_inc(cc_sem, 1)

        # ---------------- tensor (PE) ----------------
        @block.tensor
        def _(eng):
            # p-state warmup on ident while feats stream in
            eng.wait_ge(ident_sem, 16)
            for _ in range(22):
                eng.transpose(ps_tp2[:, 0:128], ident_sb[:], ident_sb[:])
            eng.wait_ge(ch_sem, 16)
            eng.wait_ge(cl_sem, 16)

            def tp_group(base):
                # 3 rotating psum buffers (ps_c is free until the first C-step)
                for t in range(base, base + 4):
                    if t >= 3:
                        eng.wait_ge(etm_sem, t - 2)
                    buf = (ps_tp, ps_tp2, ps_c)[t % 3]
                    eng.transpose(
                        buf[:, 0:128], e0t_sb[:, 128 * t : 128 * (t + 1)], ident_sb[:]
                    ).then_inc(tp_sem, 1)

            # 3-term split-bf16 aff matmul, n-outer
            for n in range(4):
                lo, hi = 512 * n, 512 * (n + 1)
                first = True
                for k in range(8):
                    if n == 0:
                        eng.wait_ge(fsl[0][k], 16)
                        eng.wait_ge(fsl[1][k], 16)
                    elif n == 1:
                        eng.wait_ge(frs[0][k], 16)
                        eng.wait_ge(frs[1][k], 16)
                    for cw, fm in ((ch_sb, fh_sb), (cl_sb, fh_sb), (ch_sb, fl_sb)):
                        mm = eng.matmul(
                            ps_aff[:, lo:hi],
                            cw[:, k, :],
                            fm[:, k, lo:hi],
                            start=first,
                            stop=(k == 7 and fm is fl_sb),
                        )
                        first = False
                mm.then_inc(affn_sem, 1)
                if n >= 1:
                    eng.wait_ge(exp_sem, n)
                    tp_group(4 * (n - 1))
            # iteration-0 R-step chunks 0..11 while exp3 lands
            eng.wait_ge(small_sem, 32)
            for t in range(12):
                eng.matmul(
                    ps_r[:, 2 * t : 2 * (t + 1)],
                    e0t_sb[:, 128 * t : 128 * (t + 1)],
                    v_sb[:],
                    start=True,
                    stop=True,
                )
            eng.wait_ge(exp_sem, 4)
            tp_group(12)
            for t in range(12, 16):
                mm = eng.matmul(
                    ps_r[:, 2 * t : 2 * (t + 1)],
                    e0t_sb[:, 128 * t : 128 * (t + 1)],
                    v_sb[:],
                    start=True,
                    stop=True,
                )
            mm.then_inc(r_sem, 1)

            for it in range(EXCH):
                # C-step
                if it == 0:
                    eng.wait_ge(etm_sem, 16)
                eng.wait_ge(w_sem, it + 1)
                for t in range(16):
                    mm = eng.matmul(
                        ps_c[:, 0:2],
                        e0tm_sb[:, t, :],
                        w_sb[:, t, :],
                        start=(t == 0),
                        stop=(t == 15),
                    )
                mm.then_inc(c_sem, 1)
                if it == EXCH - 1:
                    break
                # cross-rank reduce of gathered partials: one matvec
                eng.wait_ge(gath_sem, 16 * (it + 1))
                eng.matmul(
                    ps_tp[:, 0:1], g8p_sb[:], ones8_sb[:], start=True, stop=True
                ).then_inc(pcs_sem, 1)
                # next R-step
                eng.wait_ge(v_sem, it + 1)
                for t in range(16):
                    mm = eng.matmul(
                        ps_r[:, 2 * t : 2 * (t + 1)],
                        e0t_sb[:, 128 * t : 128 * (t + 1)],
                        v_sb[:],
                        start=True,
                        stop=True,
                    )
                mm.then_inc(r_sem, 1)

    return nc


_CACHE = {}


def _get_nc():
    if "nc" not in _CACHE:
        _CACHE["nc"] = _build_nc()
    return _CACHE["nc"]


def make_in_maps(input_features, expert_centroids):
    feats = np.ascontiguousarray(np.asarray(input_features, dtype=np.float32).reshape(-1, D))
    cent = np.asarray(expert_centroids, dtype=np.float32).reshape(SE, D)

    featsT = np.ascontiguousarray(feats.T)          # [D, N]
    centT = np.ascontiguousarray(cent.T)            # [D, SE]
    fT_h = featsT.astype(ml_dtypes.bfloat16)
    fT_l = (featsT - fT_h.astype(np.float32)).astype(ml_dtypes.bfloat16)
    cT_h = centT.astype(ml_dtypes.bfloat16)
    cT_l = (centT - cT_h.astype(np.float32)).astype(ml_dtypes.bfloat16)

    ident = np.eye(128, dtype=np.float32)
    ones8 = np.ones((8, 1), dtype=np.float32)
    v0 = np.zeros((SE, 2), np.float32)
    v0[0:64, 0] = 1.0
    v0[64:128, 1] = 1.0

    in_maps = []
    for c in range(N_CORES):
        sl = slice(TOK * c, TOK * (c + 1))
        in_maps.append(
            {
                "fh": np.ascontiguousarray(fT_h[:, sl]),
                "fl": np.ascontiguousarray(fT_l[:, sl]),
                "ch": cT_h,
                "cl": cT_l,
                "ident": ident,
                "ones8": ones8,
                "v0": v0,
            }
        )
    return in_maps


def kernel(input_features: np.ndarray, expert_centroids: np.ndarray):
    in_maps = make_in_maps(input_features, expert_centroids)
    nc = _get_nc()
    res = run_bass_kernel_spmd(nc, in_maps, list(range(N_CORES)))

    afft = np.concatenate([res.results[c]["afft"] for c in range(N_CORES)], axis=1)
    # final exchange partials are identical on every core; host does the
    # 10th R-step: V8 = 1/sum_r partials, r = ln(exp(aff) @ V8)
    g8 = np.asarray(res.results[0]["gath8"], dtype=np.float32).reshape(N_CORES, SE)
    csum = g8.sum(axis=0)
    v8 = (1.0 / csum).astype(np.float32)  # [SE]

    A = afft.reshape(KSLOT, E, N)                      # [k, e, n]
    E0 = np.exp(A.astype(np.float32))
    r = np.empty((KSLOT, N), np.float64)
    for k in range(KSLOT):
        r[k] = np.log(np.einsum('en,e->n', E0[k], v8[64 * k : 64 * (k + 1)], dtype=np.float64))
    Z = A - r[:, None, :]
    idx = np.empty((KSLOT, E, CAP), np.int32)
    vals = np.empty((KSLOT, E, CAP), np.float32)
    for k in range(KSLOT):
        for e in range(E):
            col = Z[k, e]
            part = np.sort(np.argpartition(-col, CAP - 1)[:CAP])
            order = part[np.argsort(-col[part], kind="stable")]
            idx[k, e] = order.astype(np.int32)
            vals[k, e] = A[k, e, order]
    return idx, vals
